# revision 1
# baseline (speedup 1.0000x reference)
"""Cross-attention kernel for 8 Trainium2 NeuronCores.

Problem (hardcoded): B=2, NQ=NKV=2048, QDIM=KVDIM=1024, H=16, HD=64.

Sharding: tensor-parallel over heads — 2 heads per core. Each core computes
its heads' Q/K/V projections, scores, softmax and context for the full
sequence, then an AllToAll reshards context from head-split to token-split
so the output projection is fully local; core j returns output tokens
[j*512, (j+1)*512).

All matmuls run in bf16 (fp32 PSUM accumulation). Layout trick: inputs are
fed pre-transposed ([feature, token]) so every matmul operand already has
its contraction dim on partitions — the kernel contains zero on-device
transposes. scores are computed transposed ([k, q]) so the exp'd
probabilities feed the P@V matmul directly as the stationary operand, and a
ones-column appended to V yields the softmax denominator from the same
matmul (no partition-axis reduction needed).
"""

import numpy as np
import ml_dtypes

import concourse.bass as bass
import concourse.mybir as mybir
import concourse.tile as tile
from concourse import bacc
from concourse.bass_utils import run_bass_kernel_spmd

N_CORES = 8
B = 2
NQ = NKV = 2048
C = 1024          # model dim (QDIM=KVDIM=INNER)
H, HD = 16, 64
T = B * NQ        # 4096 flattened tokens
DL = 128          # local head dims per core (2 heads * 64)
TSH = T // N_CORES  # 512 output tokens per core
SCALE = HD ** -0.5

F32 = mybir.dt.float32
BF16 = mybir.dt.bfloat16

_NC_CACHE = None
_LAST_RESULTS = None


def _build(with_collective=True, reps=None, stop_after=None):
    nc = bacc.Bacc("TRN2", target_bir_lowering=False, debug=False,
                   num_devices=N_CORES)

    qT = nc.dram_tensor("qT", [C, T], BF16, kind="ExternalInput")
    kvT = nc.dram_tensor("kvT", [C, T], BF16, kind="ExternalInput")
    wq = nc.dram_tensor("wq", [C, DL], BF16, kind="ExternalInput")
    wk = nc.dram_tensor("wk", [C, DL], BF16, kind="ExternalInput")
    wv = nc.dram_tensor("wv", [C, DL], BF16, kind="ExternalInput")
    wo = nc.dram_tensor("wo", [C, C], BF16, kind="ExternalInput")
    bias = nc.dram_tensor("bias", [C], F32, kind="ExternalInput")
    out = nc.dram_tensor("out", [TSH, C], F32, kind="ExternalOutput")

    CC = C // 128   # 8 contraction chunks
    KT = NKV // 128  # 16 k-tiles per batch
    Exp = mybir.ActivationFunctionType.Exp

    with tile.TileContext(nc) as tc:
        with (
            tc.tile_pool(name="consts", bufs=1) as consts,
            tc.tile_pool(name="xt", bufs=3) as xt,
            tc.tile_pool(name="probs", bufs=8) as probs_p,
            tc.tile_pool(name="norm", bufs=2) as norm,
            tc.tile_pool(name="outp", bufs=2) as outp,
            tc.tile_pool(name="dram", bufs=1, space="DRAM") as dram,
        ):
            # ---- constants ----
            wq_sb = consts.tile([128, CC, DL], BF16)
            nc.sync.dma_start(out=wq_sb, in_=wq.ap().rearrange("(n p) d -> p n d", p=128))
            wk_sb = consts.tile([128, CC, DL], BF16)
            nc.sync.dma_start(out=wk_sb, in_=wk.ap().rearrange("(n p) d -> p n d", p=128))
            wv_sb = consts.tile([128, CC, DL], BF16)
            nc.sync.dma_start(out=wv_sb, in_=wv.ap().rearrange("(n p) d -> p n d", p=128))
            wo_sb = consts.tile([128, CC, C], BF16)
            nc.sync.dma_start(out=wo_sb, in_=wo.ap().rearrange("(n p) e -> p n e", p=128))
            bias_sb = consts.tile([128, C], F32)
            bias_bc = bass.AP(tensor=bias, offset=0, ap=[[0, 128], [1, C]])
            nc.gpsimd.dma_start(out=bias_sb[:], in_=bias_bc)

            # persistent activations
            Kd_sb = consts.tile([128, T], BF16)   # K^T: [d_local, token]
            Qd_sb = consts.tile([128, T], BF16)   # Q^T: [d_local, token]
            # V natural [token, d] in 32 tiles of [128, 130]:
            # cols 0:64 = head0, col 64 = ones, 65:129 = head1, col 129 = ones
            V_sb = consts.tile([128, T // 128, 130], BF16)
            nc.vector.memset(V_sb[:, :, 64:65], 1.0)
            nc.vector.memset(V_sb[:, :, 129:130], 1.0)

            qT_r = qT.ap().rearrange("(n p) t -> p n t", p=128)
            kvT_r = kvT.ap().rearrange("(n p) t -> p n t", p=128)

            def _body(_it=None):
                # One shared PSUM pool for the whole body: tag "pss" slots
                # ([128,1024] = 2 banks, bufs=3) host projection / scores /
                # out-proj psums; tag "psc" (1 bank, bufs=2) hosts the two
                # per-head context accumulators. No pool-close barriers
                # between phases, so projections for batch 1 overlap the
                # ACT-bound attention of batch 0.
                with tc.tile_pool(name="ps", bufs=2, space="PSUM") as ps:

                    def proj(tt_range):
                        for tt in tt_range:
                            t0 = tt * 512
                            kvt = xt.tile([128, CC, 512], BF16, tag="kvt", name="kvt")
                            nc.sync.dma_start(out=kvt, in_=kvT_r[:, :, t0:t0 + 512])
                            qt_ = xt.tile([128, CC, 512], BF16, tag="qt", name="qt")
                            nc.sync.dma_start(out=qt_, in_=qT_r[:, :, t0:t0 + 512])

                            psk = ps.tile([128, 512], F32, tag="pss", name="psk")
                            for cc in range(CC):
                                nc.tensor.matmul(psk, lhsT=wk_sb[:, cc, :],
                                                 rhs=kvt[:, cc, :],
                                                 start=(cc == 0), stop=(cc == CC - 1))
                            nc.vector.tensor_copy(out=Kd_sb[:, t0:t0 + 512], in_=psk)

                            psq = ps.tile([128, 512], F32, tag="pss", name="psq")
                            for cc in range(CC):
                                nc.tensor.matmul(psq, lhsT=wq_sb[:, cc, :],
                                                 rhs=qt_[:, cc, :],
                                                 start=(cc == 0), stop=(cc == CC - 1))
                            nc.vector.tensor_copy(out=Qd_sb[:, t0:t0 + 512], in_=psq)

                            for s4 in range(4):
                                psv = ps.tile([128, 128], F32, tag="pss", name="psv")
                                for cc in range(CC):
                                    nc.tensor.matmul(
                                        psv, lhsT=kvt[:, cc, s4 * 128:(s4 + 1) * 128],
                                        rhs=wv_sb[:, cc, :],
                                        start=(cc == 0), stop=(cc == CC - 1))
                                ti = tt * 4 + s4
                                # one strided copy fills both head blocks
                                # (cols 0:64 and 65:129), skipping the ones
                                # columns: out free pattern [2 (stride 65), 64]
                                vdst = V_sb[:, ti, 0:64]
                                vdst2 = bass.AP(
                                    tensor=vdst.tensor, offset=vdst.offset,
                                    ap=[vdst.ap[0], [65, 2], [1, 64]])
                                nc.vector.tensor_copy(
                                    out=vdst2,
                                    in_=psv[:].rearrange("p (g x) -> p g x", g=2))

                    def attn_group(b, qv, psc, kt_range):
                        # Software-pipelined emission: scores(kt+1) is placed
                        # BEFORE pv(kt) in the (in-order) PE stream, so the
                        # PE never stalls on exp(kt) before issuing the next
                        # scores pair -- keeps the scalar engine (exp, the
                        # bottleneck) fed back-to-back.
                        q0 = b * NQ + qv * 512

                        def scores(kt):
                            k0 = b * NKV + kt * 128
                            # both heads' transposed scores into one
                            # 2-bank tile -> a single wide exp
                            pair = ps.tile([128, 1024], F32, tag="pss", name="pair")
                            for h in range(2):
                                hs = slice(h * 64, (h + 1) * 64)
                                nc.tensor.matmul(
                                    pair[:, h * 512:(h + 1) * 512],
                                    lhsT=Kd_sb[hs, k0:k0 + 128],
                                    rhs=Qd_sb[hs, q0:q0 + 512],
                                    start=True, stop=True)
                            return pair

                        def pv(kt, pr):
                            vt = b * KT + kt
                            for h in range(2):
                                nc.tensor.matmul(
                                    psc[h],
                                    lhsT=V_sb[:, vt, h * 65:(h + 1) * 65],
                                    rhs=pr[:, h * 512:(h + 1) * 512],
                                    start=(kt == 0), stop=(kt == KT - 1))

                        kts = list(kt_range)
                        pair = scores(kts[0])
                        for n, kt in enumerate(kts):
                            pr = probs_p.tile([128, 1024], BF16, tag="probs",
                                              name="pr")
                            nc.scalar.activation(out=pr, in_=pair, func=Exp,
                                                 scale=SCALE)
                            if n + 1 < len(kts):
                                pair = scores(kts[n + 1])
                            pv(kt, pr)

                    def attn_norm(b, qv, psc, a2a_in):
                        q0 = b * NQ + qv * 512
                        j = q0 // TSH
                        for h in range(2):
                            recip = norm.tile([1, 512], F32, tag="recip", name="recip")
                            nc.vector.reciprocal(out=recip, in_=psc[h][64:65, :])
                            bc = norm.tile([64, 512], F32, tag="bc", name="bc")
                            nc.gpsimd.partition_broadcast(bc[:], recip[:])
                            ctxn = norm.tile([64, 512], BF16, tag="ctxn", name="ctxn")
                            nc.vector.tensor_mul(ctxn, psc[h][0:64, :], bc)
                            nc.sync.dma_start(out=a2a_in[j, h * 64:(h + 1) * 64, :],
                                              in_=ctxn)

                    def alloc_psc():
                        return [ps.tile([65, 512], F32, tag="psc", name=f"psc{_h}",
                                        bufs=4) for _h in range(2)]

                    a2a_in = dram.tile([N_CORES, DL, TSH], BF16)
                    a2a_out = dram.tile([N_CORES, DL, TSH], BF16)

                    # batch-0 lead-in: interleave each projection t-tile
                    # with a 4-kt slice of the first attention q-tile so the
                    # scalar engine (exp, the bottleneck) starts early.
                    proj(range(0, 1))
                    psc0 = alloc_psc()
                    for tt in range(1, 4):
                        attn_group(0, 0, psc0, range((tt - 1) * 4, tt * 4))
                        proj(range(tt, tt + 1))
                    attn_group(0, 0, psc0, range(12, 16))
                    attn_norm(0, 0, psc0, a2a_in)
                    if stop_after == "proj":
                        return
                    # batch-0 qv1-3 with batch-1 projections threaded in,
                    # then batch-1 lead-in interleaved the same way as batch 0
                    psc_ = alloc_psc()
                    attn_group(0, 1, psc_, range(KT))
                    attn_norm(0, 1, psc_, a2a_in)
                    proj(range(4, 5))
                    psc_ = alloc_psc()
                    attn_group(0, 2, psc_, range(KT))
                    attn_norm(0, 2, psc_, a2a_in)
                    proj(range(5, 6))
                    psc_ = alloc_psc()
                    attn_group(0, 3, psc_, range(KT))
                    attn_norm(0, 3, psc_, a2a_in)
                    psc1 = alloc_psc()
                    attn_group(1, 0, psc1, range(0, 4))
                    proj(range(6, 7))
                    attn_group(1, 0, psc1, range(4, 8))
                    proj(range(7, 8))
                    attn_group(1, 0, psc1, range(8, 16))
                    attn_norm(1, 0, psc1, a2a_in)
                    for qv in range(1, 4):
                        psc_ = alloc_psc()
                        attn_group(1, qv, psc_, range(KT))
                        attn_norm(1, qv, psc_, a2a_in)

                    if stop_after == "attn":
                        return
                    if with_collective:
                        nc.gpsimd.collective_compute(
                            "AllToAll", mybir.AluOpType.bypass,
                            replica_groups=[list(range(N_CORES))],
                            ins=[a2a_in.opt()], outs=[a2a_out.opt()])
                    else:
                        a2a_out = a2a_in  # timing-sim variant: skip collective

                    # ---- output projection (local tokens only) ----
                    ctxF = outp.tile([128, N_CORES, TSH], BF16)
                    for i in range(N_CORES):
                        nc.sync.dma_start(out=ctxF[:, i, :], in_=a2a_out[i])
                    for m in range(TSH // 128):
                        ob = outp.tile([128, C], F32, tag="ob", name="ob")
                        for half in range(2):
                            pso = ps.tile([128, 512], F32, tag="pss", name="pso")
                            for i in range(N_CORES):
                                nc.tensor.matmul(
                                    pso, lhsT=ctxF[:, i, m * 128:(m + 1) * 128],
                                    rhs=wo_sb[:, i, half * 512:(half + 1) * 512],
                                    start=(i == 0), stop=(i == N_CORES - 1))
                            nc.vector.tensor_add(ob[:, half * 512:(half + 1) * 512],
                                                 pso,
                                                 bias_sb[:, half * 512:(half + 1) * 512])
                        nc.sync.dma_start(out=out.ap()[m * 128:(m + 1) * 128, :], in_=ob)

            if reps is None:
                _body()
            else:
                with tc.For_i(0, reps, 1) as _it:
                    _body(_it)
    nc.compile()
    return nc


def _get_nc():
    global _NC_CACHE
    if _NC_CACHE is None:
        _NC_CACHE = _build()
    return _NC_CACHE


def prep_in_maps(query, key_value, w_q, w_kv, w_out, b_out):
    bf = ml_dtypes.bfloat16
    q2 = np.asarray(query, np.float32).reshape(T, C)
    kv2 = np.asarray(key_value, np.float32).reshape(T, C)
    qT = np.ascontiguousarray(q2.T).astype(bf)
    kvT = np.ascontiguousarray(kv2.T).astype(bf)
    wo = np.asarray(w_out, np.float32).astype(bf)
    bias = np.asarray(b_out, np.float32)

    in_maps = []
    for j in range(N_CORES):
        cs = slice(j * DL, (j + 1) * DL)
        in_maps.append({
            "qT": qT,
            "kvT": kvT,
            "wq": np.ascontiguousarray(np.asarray(w_q, np.float32)[:, cs]).astype(bf),
            "wk": np.ascontiguousarray(np.asarray(w_kv, np.float32)[:, cs]).astype(bf),
            "wv": np.ascontiguousarray(
                np.asarray(w_kv, np.float32)[:, C + j * DL: C + (j + 1) * DL]).astype(bf),
            "wo": wo,
            "bias": bias,
        })
    return in_maps


def kernel(query, key_value, w_q, w_kv, w_out, b_out):
    global _LAST_RESULTS
    in_maps = prep_in_maps(query, key_value, w_q, w_kv, w_out, b_out)
    nc = _get_nc()
    res = run_bass_kernel_spmd(nc, in_maps, core_ids=list(range(N_CORES)))
    _LAST_RESULTS = res
    full = np.concatenate([res.results[j]["out"] for j in range(N_CORES)], axis=0)
    return full.reshape(B, NQ, C)



# revision 34
# speedup vs baseline: 9.8792x; 9.8792x over previous
"""Cross-attention kernel for 8 Trainium2 NeuronCores.

Problem (hardcoded): B=2, NQ=NKV=2048, QDIM=KVDIM=1024, H=16, HD=64.

Sharding: tensor-parallel over heads — 2 heads per core. Each core computes
its heads' Q/K/V projections, scores, softmax and context for the full
sequence, then an AllToAll reshards context from head-split to token-split
so the output projection is fully local; core j returns output tokens
[j*512, (j+1)*512).

All matmuls run in bf16 (fp32 PSUM accumulation). Inputs are fed
pre-transposed ([feature, token]) so every matmul operand already has its
contraction dim on partitions. scores are computed transposed ([k, q]) as
two row-tiled matmuls (head0 rows 0:64, head1 rows 64:128) which execute
concurrently in distinct PE row-groups; the exp'd probabilities feed the
P@V matmul directly as the moving operand, and a ones-column appended to V
yields the softmax denominator from the same matmul. V is produced in
[d, token] layout by stationary-weight matmuls (full-rate, N=512) and
moved to the [token, d] layout PV needs via XBAR transpose DMAs.

The attention stream is software-pipelined: pv lags scores/exp by 2
k-tiles so the in-order PE stream never starves the scalar engine (exp is
the bottleneck engine); each group's normalization (finish) is emitted
inside the next group's pipeline.
"""

import numpy as np
import ml_dtypes

import concourse.bass as bass
import concourse.mybir as mybir
import concourse.tile as tile
from concourse import bacc
from concourse.bass_utils import run_bass_kernel_spmd

N_CORES = 8
B = 2
NQ = NKV = 2048
C = 1024          # model dim (QDIM=KVDIM=INNER)
H, HD = 16, 64
T = B * NQ        # 4096 flattened tokens
DL = 128          # local head dims per core (2 heads * 64)
TSH = T // N_CORES  # 512 output tokens per core
SCALE = HD ** -0.5

F32 = mybir.dt.float32
BF16 = mybir.dt.bfloat16
FP8 = mybir.dt.float8e4

_NC_CACHE = None
_LAST_RESULTS = None


def _build(with_collective=True, reps=None, stop_after=None):
    nc = bacc.Bacc("TRN2", target_bir_lowering=False, debug=False,
                   num_devices=N_CORES)

    qT = nc.dram_tensor("qT", [C, T], BF16, kind="ExternalInput")
    kvT = nc.dram_tensor("kvT", [C, T], BF16, kind="ExternalInput")
    wq = nc.dram_tensor("wq", [C, DL], BF16, kind="ExternalInput")
    wk = nc.dram_tensor("wk", [C, DL], BF16, kind="ExternalInput")
    wv = nc.dram_tensor("wv", [C, DL], BF16, kind="ExternalInput")
    wo = nc.dram_tensor("wo", [C, C], BF16, kind="ExternalInput")
    bias = nc.dram_tensor("bias", [C], F32, kind="ExternalInput")
    out = nc.dram_tensor("out", [TSH, C], F32, kind="ExternalOutput")

    CC = C // 128   # 8 contraction chunks
    KT = NKV // 128  # 16 k-tiles per batch
    VW = 144        # V_sb row (fp8): h0 V 0:64 ones 64 | h1 V 72:136 ones 136
    Exp = mybir.ActivationFunctionType.Exp

    with tile.TileContext(nc) as tc:
        with (
            tc.tile_pool(name="consts", bufs=1) as consts,
            tc.tile_pool(name="xt", bufs=3) as xt,
            tc.tile_pool(name="vt", bufs=2) as vtp,
            tc.tile_pool(name="probs", bufs=8) as probs_p,
            tc.tile_pool(name="norm", bufs=2) as norm,
            tc.tile_pool(name="outp", bufs=2) as outp,
            tc.tile_pool(name="dram", bufs=1, space="DRAM") as dram,
        ):
            # ---- constants; wq/wk first on the SP queue (first proj tile
            # needs them), wv/wo/bias on the SWDGE queue in the background
            wq_sb = consts.tile([128, CC, DL], BF16)
            nc.sync.dma_start(out=wq_sb, in_=wq.ap().rearrange("(n p) d -> p n d", p=128))
            wk_sb = consts.tile([128, CC, DL], BF16)
            nc.sync.dma_start(out=wk_sb, in_=wk.ap().rearrange("(n p) d -> p n d", p=128))
            wv_sb = consts.tile([128, CC, DL], BF16)
            nc.sync.dma_start(out=wv_sb, in_=wv.ap().rearrange("(n p) d -> p n d", p=128))
            wo_sb = consts.tile([128, CC, C], BF16)
            nc.sync.dma_start(out=wo_sb, in_=wo.ap().rearrange("(n p) e -> p n e", p=128))
            bias_sb = consts.tile([128, C], F32)
            bias_bc = bass.AP(tensor=bias, offset=0, ap=[[0, 128], [1, C]])
            nc.gpsimd.dma_start(out=bias_sb[:], in_=bias_bc)

            # persistent activations
            Kd_sb = consts.tile([128, T], BF16)   # K^T: [d_local, token]
            Qd_sb = consts.tile([128, T], BF16)   # Q^T: [d_local, token]
            V_sb = consts.tile([128, T // 128, VW], FP8)  # V natural [tok, d]
            nc.vector.memset(V_sb[:, :, 64:65], 1.0)
            nc.vector.memset(V_sb[:, :, 136:137], 1.0)

            qT_r = qT.ap().rearrange("(n p) t -> p n t", p=128)
            kvT_r = kvT.ap().rearrange("(n p) t -> p n t", p=128)

            def _body(_it=None):
                # PSUM budget (8 banks): pair 2x[128,1024]=4, psc 2x[65,512]=2,
                # bg 2x[128,512]=2. "bg" hosts projection and out-projection
                # psums so they never steal the scores/exp double-buffer.
                with tc.tile_pool(name="ps", bufs=2, space="PSUM") as ps:

                    a2a_in = dram.tile([N_CORES, DL, TSH], BF16)
                    a2a_out = dram.tile([N_CORES, DL, TSH], BF16)

                    def proj(tt):
                        t0 = tt * 512
                        kvt = xt.tile([128, CC, 512], BF16, tag="kvt", name="kvt")
                        nc.sync.dma_start(out=kvt, in_=kvT_r[:, :, t0:t0 + 512])
                        qt_ = xt.tile([128, CC, 512], BF16, tag="qt", name="qt")
                        nc.sync.dma_start(out=qt_, in_=qT_r[:, :, t0:t0 + 512])

                        psk = ps.tile([128, 512], F32, tag="pss", name="psk")
                        for cc in range(CC):
                            nc.tensor.matmul(psk, lhsT=wk_sb[:, cc, :],
                                             rhs=kvt[:, cc, :],
                                             start=(cc == 0), stop=(cc == CC - 1))
                        nc.vector.tensor_copy(out=Kd_sb[:, t0:t0 + 512], in_=psk)

                        psq = ps.tile([128, 512], F32, tag="pss", name="psq")
                        for cc in range(CC):
                            nc.tensor.matmul(psq, lhsT=wq_sb[:, cc, :],
                                             rhs=qt_[:, cc, :],
                                             start=(cc == 0), stop=(cc == CC - 1))
                        nc.vector.tensor_copy(out=Qd_sb[:, t0:t0 + 512], in_=psq)

                        # V natural [tok, d]: kvt-chunk-stationary matmuls per
                        # 128-token block, strided DVE copy into both heads
                        for s4 in range(4):
                            psv = ps.tile([128, 128], F32, tag="pss", name="psv")
                            for cc in range(CC):
                                nc.tensor.matmul(
                                    psv, lhsT=kvt[:, cc, s4 * 128:(s4 + 1) * 128],
                                    rhs=wv_sb[:, cc, :],
                                    start=(cc == 0), stop=(cc == CC - 1))
                            ti = tt * 4 + s4
                            vdst = V_sb[:, ti, 0:64]
                            vdst2 = bass.AP(
                                tensor=vdst.tensor, offset=vdst.offset,
                                ap=[vdst.ap[0], [72, 2], [1, 64]])
                            nc.vector.tensor_copy(
                                out=vdst2,
                                in_=psv[:].rearrange("p (g x) -> p g x", g=2))

                    def attn_group(b, qv, hooks=None):
                        # pv lags scores/exp by PV_LAG kts. hooks[kt] (the
                        # previous group's finish, or a projection t-tile) is
                        # emitted after exp(kt); psc is allocated lazily AFTER
                        # the kt=1 hook so the previous group's finish has
                        # registered its psc reads first (2-slot psc tag).
                        hooks = hooks or {}
                        q0 = b * NQ + qv * 512
                        psc = None

                        def scores(kt):
                            k0 = b * NKV + kt * 128
                            pair = ps.tile([128, 1024], F32, tag="pss", name="pair")
                            for h in range(2):
                                hs = slice(h * 64, (h + 1) * 64)
                                nc.tensor.matmul(
                                    pair[:, h * 512:(h + 1) * 512],
                                    lhsT=Kd_sb[hs, k0:k0 + 128],
                                    rhs=Qd_sb[hs, q0:q0 + 512],
                                    start=True, stop=True)
                            return pair

                        def pv2(dkt, prd):
                            vtb = b * KT + 2 * dkt
                            for h in range(2):
                                nc.tensor.matmul(
                                    psc[h],
                                    lhsT=V_sb[:, vtb:vtb + 2,
                                              h * 72:h * 72 + 65],
                                    rhs=prd[:, :, h, :],
                                    start=(dkt == 0), stop=(dkt == KT // 2 - 1),
                                    perf_mode=mybir.MatmulPerfMode.DoubleRow)

                        pend = []
                        prd = None
                        pair = scores(0)
                        for kt in range(KT):
                            if kt % 2 == 0:
                                prd = probs_p.tile([128, 2, 2, 512], FP8,
                                                   tag="probs", name="prd")
                            nc.scalar.activation(out=prd[:, kt % 2], in_=pair,
                                                 func=Exp, scale=SCALE)
                            # hooks BEFORE scores(kt+1): a hook that projects
                            # tile p must be emitted before any scores that
                            # read it
                            if kt in hooks:
                                hooks[kt]()
                            if kt + 1 < KT:
                                pair = scores(kt + 1)
                            if kt % 2 == 1:
                                pend.append((kt // 2, prd))
                                if len(pend) > 1:
                                    if psc is None:
                                        psc = [ps.tile([65, 512], F32, tag="psc",
                                                       name=f"psc{_h}", bufs=4)
                                               for _h in range(2)]
                                    pv2(*pend.pop(0))
                        for item in pend:
                            pv2(*item)
                        return psc

                    def finish(b, qv, psc):
                        # normalize ctx by the softmax denominator (psc row 64)
                        # and stage it for the AllToAll: dest core b*4+qv.
                        j = b * 4 + qv
                        for h in range(2):
                            recip = norm.tile([1, 512], F32, tag=f"recip{h}",
                                              name="recip")
                            nc.vector.reciprocal(out=recip, in_=psc[h][64:65, :])
                            bcf = norm.tile([64, 512], F32, tag=f"bcf{h}",
                                            name="bcf")
                            nc.gpsimd.partition_broadcast(bcf[:], recip[:])
                            ctxn = norm.tile([64, 512], BF16, tag=f"ctxn{h}",
                                             name="ctxn")
                            nc.vector.tensor_mul(ctxn, psc[h][0:64, :], bcf)
                            nc.sync.dma_start(out=a2a_in[j, h * 64:(h + 1) * 64, :],
                                              in_=ctxn)

                    # Schedule: each group's finish is emitted at hook kt=1 of
                    # the NEXT group (two scores/exp pairs already in flight);
                    # projection t-tiles are threaded in where they are needed.
                    proj(0)
                    g = {}
                    g[0, 0] = attn_group(0, 0, {3: lambda: proj(1),
                                                7: lambda: proj(2),
                                                11: lambda: proj(3)})
                    if stop_after == "proj":
                        return
                    g[0, 1] = attn_group(0, 1, {1: lambda: finish(0, 0, g[0, 0])})
                    g[0, 2] = attn_group(0, 2, {1: lambda: finish(0, 1, g[0, 1]),
                                                7: lambda: proj(4)})
                    g[0, 3] = attn_group(0, 3, {1: lambda: finish(0, 2, g[0, 2]),
                                                7: lambda: proj(5)})
                    g[1, 0] = attn_group(1, 0, {1: lambda: finish(0, 3, g[0, 3]),
                                                3: lambda: proj(6),
                                                7: lambda: proj(7)})
                    g[1, 1] = attn_group(1, 1, {1: lambda: finish(1, 0, g[1, 0])})
                    g[1, 2] = attn_group(1, 2, {1: lambda: finish(1, 1, g[1, 1])})
                    g[1, 3] = attn_group(1, 3, {1: lambda: finish(1, 2, g[1, 2])})
                    finish(1, 3, g[1, 3])

                    if stop_after == "attn":
                        return
                    if with_collective:
                        nc.gpsimd.collective_compute(
                            "AllToAll", mybir.AluOpType.bypass,
                            replica_groups=[list(range(N_CORES))],
                            ins=[a2a_in.opt()], outs=[a2a_out.opt()])
                    else:
                        a2a_out = a2a_in  # timing-sim variant: skip collective

                    # ---- output projection (local tokens only) ----
                    ctxF = outp.tile([128, N_CORES, TSH], BF16)
                    for i in range(N_CORES):
                        nc.sync.dma_start(out=ctxF[:, i, :], in_=a2a_out[i])
                    for m in range(TSH // 128):
                        ob = outp.tile([128, C], F32, tag="ob", name="ob")
                        for half in range(2):
                            pso = ps.tile([128, 512], F32, tag="pss", name="pso")
                            for i in range(N_CORES):
                                nc.tensor.matmul(
                                    pso, lhsT=ctxF[:, i, m * 128:(m + 1) * 128],
                                    rhs=wo_sb[:, i, half * 512:(half + 1) * 512],
                                    start=(i == 0), stop=(i == N_CORES - 1))
                            nc.vector.tensor_add(ob[:, half * 512:(half + 1) * 512],
                                                 pso,
                                                 bias_sb[:, half * 512:(half + 1) * 512])
                        nc.sync.dma_start(out=out.ap()[m * 128:(m + 1) * 128, :], in_=ob)

            if reps is None:
                _body()
            else:
                with tc.For_i(0, reps, 1) as _it:
                    _body(_it)
    nc.compile()
    return nc


def _get_nc():
    global _NC_CACHE
    if _NC_CACHE is None:
        _NC_CACHE = _build()
    return _NC_CACHE


def prep_in_maps(query, key_value, w_q, w_kv, w_out, b_out):
    bf = ml_dtypes.bfloat16
    q2 = np.asarray(query, np.float32).reshape(T, C)
    kv2 = np.asarray(key_value, np.float32).reshape(T, C)
    qT_ = np.ascontiguousarray(q2.T).astype(bf)
    kvT_ = np.ascontiguousarray(kv2.T).astype(bf)
    wo_ = np.asarray(w_out, np.float32).astype(bf)
    bias = np.asarray(b_out, np.float32)

    in_maps = []
    for j in range(N_CORES):
        cs = slice(j * DL, (j + 1) * DL)
        in_maps.append({
            "qT": qT_,
            "kvT": kvT_,
            "wq": np.ascontiguousarray(np.asarray(w_q, np.float32)[:, cs]).astype(bf),
            "wk": np.ascontiguousarray(np.asarray(w_kv, np.float32)[:, cs]).astype(bf),
            "wv": np.ascontiguousarray(
                np.asarray(w_kv, np.float32)[:, C + j * DL: C + (j + 1) * DL]).astype(bf),
            "wo": wo_,
            "bias": bias,
        })
    return in_maps


def kernel(query, key_value, w_q, w_kv, w_out, b_out):
    global _LAST_RESULTS
    in_maps = prep_in_maps(query, key_value, w_q, w_kv, w_out, b_out)
    nc = _get_nc()
    res = run_bass_kernel_spmd(nc, in_maps, core_ids=list(range(N_CORES)))
    _LAST_RESULTS = res
    full = np.concatenate([res.results[j]["out"] for j in range(N_CORES)], axis=0)
    return full.reshape(B, NQ, C)


# revision 37
# speedup vs baseline: 12.0015x; 1.2148x over previous
"""Cross-attention kernel for 8 Trainium2 NeuronCores.

Problem (hardcoded): B=2, NQ=NKV=2048, QDIM=KVDIM=1024, H=16, HD=64.

Sharding: tensor-parallel over heads — 2 heads per core. Each core computes
its heads' Q/K/V projections, scores, softmax and context for the full
sequence, then an AllToAll reshards context from head-split to token-split
so the output projection is fully local; core j returns output tokens
[j*512, (j+1)*512).

All matmuls run in bf16 (fp32 PSUM accumulation). Inputs are fed
pre-transposed ([feature, token]) so every matmul operand already has its
contraction dim on partitions. scores are computed transposed ([k, q]) as
two row-tiled matmuls (head0 rows 0:64, head1 rows 64:128) which execute
concurrently in distinct PE row-groups; the exp'd probabilities feed the
P@V matmul directly as the moving operand, and a ones-column appended to V
yields the softmax denominator from the same matmul.

The attention stream is software-pipelined: pv lags scores/exp by 2
k-tiles so the in-order PE stream never starves the scalar engine (exp is
the bottleneck engine); each group's normalization (finish) is emitted
inside the next group's pipeline. Projection work is threaded into the
pipeline at hook points: each K/V tile uses only TWO psum allocations (K
in half of a 2-bank tile shared with nothing else, V's four 128-token
blocks in one bank) so the scores-pair psum round-robin is barely
disturbed, and each Q tile is projected lazily in the group just before
the one that consumes it, keeping the critical lead-in (first q-tile +
batch-0 K/V) as light as possible.
"""

import numpy as np
import ml_dtypes

import concourse.bass as bass
import concourse.mybir as mybir
import concourse.tile as tile
from concourse import bacc
from concourse.bass_utils import run_bass_kernel_spmd

N_CORES = 8
B = 2
NQ = NKV = 2048
C = 1024          # model dim (QDIM=KVDIM=INNER)
H, HD = 16, 64
T = B * NQ        # 4096 flattened tokens
DL = 128          # local head dims per core (2 heads * 64)
TSH = T // N_CORES  # 512 output tokens per core
SCALE = HD ** -0.5

F32 = mybir.dt.float32
BF16 = mybir.dt.bfloat16

_NC_CACHE = None
_LAST_RESULTS = None


def _build(with_collective=True, reps=None, stop_after=None):
    nc = bacc.Bacc("TRN2", target_bir_lowering=False, debug=False,
                   num_devices=N_CORES)

    qT = nc.dram_tensor("qT", [C, T], BF16, kind="ExternalInput")
    kvT = nc.dram_tensor("kvT", [C, T], BF16, kind="ExternalInput")
    wq = nc.dram_tensor("wq", [C, DL], BF16, kind="ExternalInput")
    wk = nc.dram_tensor("wk", [C, DL], BF16, kind="ExternalInput")
    wv = nc.dram_tensor("wv", [C, DL], BF16, kind="ExternalInput")
    wo = nc.dram_tensor("wo", [C, C], BF16, kind="ExternalInput")
    bias = nc.dram_tensor("bias", [C], F32, kind="ExternalInput")
    out = nc.dram_tensor("out", [TSH, C], F32, kind="ExternalOutput")

    CC = C // 128   # 8 contraction chunks
    KT = NKV // 128  # 16 k-tiles per batch
    VW = 136        # V_sb row: h0 V 0:64 ones 64 | h1 V 68:132 ones 132
    Exp = mybir.ActivationFunctionType.Exp

    with tile.TileContext(nc) as tc:
        with (
            tc.tile_pool(name="consts", bufs=1) as consts,
            tc.tile_pool(name="xt", bufs=3) as xt,
            tc.tile_pool(name="vt", bufs=2) as vtp,
            tc.tile_pool(name="probs", bufs=8) as probs_p,
            tc.tile_pool(name="norm", bufs=2) as norm,
            tc.tile_pool(name="outp", bufs=2) as outp,
            tc.tile_pool(name="dram", bufs=1, space="DRAM") as dram,
        ):
            # ---- constants; wq/wk first on the SP queue (first proj tile
            # needs them), wv/wo/bias on the SWDGE queue in the background
            wq_sb = consts.tile([128, CC, DL], BF16)
            nc.sync.dma_start(out=wq_sb, in_=wq.ap().rearrange("(n p) d -> p n d", p=128))
            wk_sb = consts.tile([128, CC, DL], BF16)
            nc.sync.dma_start(out=wk_sb, in_=wk.ap().rearrange("(n p) d -> p n d", p=128))
            wv_sb = consts.tile([128, CC, DL], BF16)
            nc.sync.dma_start(out=wv_sb, in_=wv.ap().rearrange("(n p) d -> p n d", p=128))
            wo_sb = consts.tile([128, CC, C], BF16)
            nc.sync.dma_start(out=wo_sb, in_=wo.ap().rearrange("(n p) e -> p n e", p=128))
            bias_sb = consts.tile([128, C], F32)
            bias_bc = bass.AP(tensor=bias, offset=0, ap=[[0, 128], [1, C]])
            nc.gpsimd.dma_start(out=bias_sb[:], in_=bias_bc)

            # persistent activations
            Kd_sb = consts.tile([128, T], BF16)   # K^T: [d_local, token]
            Qd_sb = consts.tile([128, T], BF16)   # Q^T: [d_local, token]
            V_sb = consts.tile([128, T // 128, VW], BF16)  # V natural [tok, d]
            nc.vector.memset(V_sb[:, :, 64:65], 1.0)
            nc.vector.memset(V_sb[:, :, 132:133], 1.0)

            qT_r = qT.ap().rearrange("(n p) t -> p n t", p=128)
            kvT_r = kvT.ap().rearrange("(n p) t -> p n t", p=128)

            def _body(_it=None):
                # PSUM budget (8 banks): pair 2x[128,1024]=4, psc 2x[65,512]=2,
                # bg 2x[128,512]=2. "bg" hosts projection and out-projection
                # psums so they never steal the scores/exp double-buffer.
                with tc.tile_pool(name="ps", bufs=2, space="PSUM") as ps:

                    a2a_in = dram.tile([N_CORES, DL, TSH], BF16)
                    a2a_out = dram.tile([N_CORES, DL, TSH], BF16)

                    def proj_q(tt):
                        t0 = tt * 512
                        qt_ = xt.tile([128, CC, 512], BF16, tag="qt", name="qt")
                        nc.sync.dma_start(out=qt_, in_=qT_r[:, :, t0:t0 + 512])
                        psq = ps.tile([128, 512], F32, tag="pss", name="psq")
                        for cc in range(CC):
                            nc.tensor.matmul(psq, lhsT=wq_sb[:, cc, :],
                                             rhs=qt_[:, cc, :],
                                             start=(cc == 0), stop=(cc == CC - 1))
                        nc.vector.tensor_copy(out=Qd_sb[:, t0:t0 + 512], in_=psq)

                    def proj(tt):
                        t0 = tt * 512
                        kvt = xt.tile([128, CC, 512], BF16, tag="kvt", name="kvt")
                        nc.sync.dma_start(out=kvt, in_=kvT_r[:, :, t0:t0 + 512])

                        # K and Q share one 2-bank psum tile, V's four
                        # 128-token blocks share one bank: 2 pss allocations
                        # per tile instead of 6, so the scores-pair round-robin
                        # is disturbed at most twice per projection tile.
                        pkq = ps.tile([128, 1024], F32, tag="pss", name="pkq")
                        for cc in range(CC):
                            nc.tensor.matmul(pkq[:, 0:512], lhsT=wk_sb[:, cc, :],
                                             rhs=kvt[:, cc, :],
                                             start=(cc == 0), stop=(cc == CC - 1))
                        nc.vector.tensor_copy(out=Kd_sb[:, t0:t0 + 512],
                                              in_=pkq[:, 0:512])

                        # V natural [tok, d]: kvt-chunk-stationary matmuls per
                        # 128-token block, strided DVE copy into both heads
                        psv4 = ps.tile([128, 512], F32, tag="pss", name="psv4")
                        for s4 in range(4):
                            for cc in range(CC):
                                nc.tensor.matmul(
                                    psv4[:, s4 * 128:(s4 + 1) * 128],
                                    lhsT=kvt[:, cc, s4 * 128:(s4 + 1) * 128],
                                    rhs=wv_sb[:, cc, :],
                                    start=(cc == 0), stop=(cc == CC - 1))
                        for s4 in range(4):
                            ti = tt * 4 + s4
                            vdst = V_sb[:, ti, 0:64]
                            vdst2 = bass.AP(
                                tensor=vdst.tensor, offset=vdst.offset,
                                ap=[vdst.ap[0], [68, 2], [1, 64]])
                            nc.vector.tensor_copy(
                                out=vdst2,
                                in_=psv4[:, s4 * 128:(s4 + 1) * 128].rearrange(
                                    "p (g x) -> p g x", g=2))

                    def attn_group(b, qv, hooks=None):
                        # pv lags scores/exp by PV_LAG kts. hooks[kt] (the
                        # previous group's finish, or a projection t-tile) is
                        # emitted after exp(kt); psc is allocated lazily AFTER
                        # the kt=1 hook so the previous group's finish has
                        # registered its psc reads first (2-slot psc tag).
                        hooks = hooks or {}
                        q0 = b * NQ + qv * 512
                        psc = None

                        def scores(kt):
                            k0 = b * NKV + kt * 128
                            pair = ps.tile([128, 1024], F32, tag="pss", name="pair")
                            for h in range(2):
                                hs = slice(h * 64, (h + 1) * 64)
                                nc.tensor.matmul(
                                    pair[:, h * 512:(h + 1) * 512],
                                    lhsT=Kd_sb[hs, k0:k0 + 128],
                                    rhs=Qd_sb[hs, q0:q0 + 512],
                                    start=True, stop=True)
                            return pair

                        def pv(kt, pr):
                            vt_ = b * KT + kt
                            for h in range(2):
                                nc.tensor.matmul(
                                    psc[h],
                                    lhsT=V_sb[:, vt_, h * 68:h * 68 + 65],
                                    rhs=pr[:, h * 512:(h + 1) * 512],
                                    start=(kt == 0), stop=(kt == KT - 1))

                        PV_LAG = 2
                        pend = []
                        pair = scores(0)
                        for kt in range(KT):
                            pr = probs_p.tile([128, 1024], BF16, tag="probs",
                                              name="pr")
                            nc.scalar.activation(out=pr, in_=pair, func=Exp,
                                                 scale=SCALE)
                            pend.append((kt, pr))
                            # hooks BEFORE scores(kt+1): a hook that projects
                            # tile p must be emitted before any scores that
                            # read it
                            if kt in hooks:
                                hooks[kt]()
                            if kt + 1 < KT:
                                pair = scores(kt + 1)
                            if kt + 1 >= PV_LAG:
                                if psc is None:
                                    psc = [ps.tile([65, 512], F32, tag="psc",
                                                   name=f"psc{_h}", bufs=4)
                                           for _h in range(2)]
                                pv(*pend.pop(0))
                        for item in pend:
                            pv(*item)
                        return psc

                    def finish(b, qv, psc):
                        # normalize ctx by the softmax denominator (psc row 64)
                        # and stage it for the AllToAll: dest core b*4+qv.
                        j = b * 4 + qv
                        for h in range(2):
                            recip = norm.tile([1, 512], F32, tag=f"recip{h}",
                                              name="recip")
                            nc.vector.reciprocal(out=recip, in_=psc[h][64:65, :])
                            bcf = norm.tile([64, 512], F32, tag=f"bcf{h}",
                                            name="bcf")
                            nc.gpsimd.partition_broadcast(bcf[:], recip[:])
                            ctxn = norm.tile([64, 512], BF16, tag=f"ctxn{h}",
                                             name="ctxn")
                            nc.vector.tensor_mul(ctxn, psc[h][0:64, :], bcf)
                            nc.sync.dma_start(out=a2a_in[j, h * 64:(h + 1) * 64, :],
                                              in_=ctxn)

                    # Schedule: each group's finish is emitted at hook kt=1 of
                    # the NEXT group (two scores/exp pairs already in flight);
                    # projection t-tiles are threaded in where they are needed.
                    proj(0)
                    proj_q(0)
                    g = {}
                    g[0, 0] = attn_group(0, 0, {3: lambda: proj(1),
                                                7: lambda: proj(2),
                                                11: lambda: proj(3),
                                                13: lambda: proj_q(1)})
                    if stop_after == "proj":
                        return
                    g[0, 1] = attn_group(0, 1, {1: lambda: finish(0, 0, g[0, 0]),
                                                7: lambda: proj_q(2)})
                    g[0, 2] = attn_group(0, 2, {1: lambda: finish(0, 1, g[0, 1]),
                                                5: lambda: proj_q(3),
                                                9: lambda: proj(4)})
                    g[0, 3] = attn_group(0, 3, {1: lambda: finish(0, 2, g[0, 2]),
                                                5: lambda: proj(5),
                                                11: lambda: proj_q(4)})
                    g[1, 0] = attn_group(1, 0, {1: lambda: finish(0, 3, g[0, 3]),
                                                3: lambda: proj(6),
                                                7: lambda: proj(7),
                                                11: lambda: proj_q(5)})
                    g[1, 1] = attn_group(1, 1, {1: lambda: finish(1, 0, g[1, 0]),
                                                7: lambda: proj_q(6)})
                    g[1, 2] = attn_group(1, 2, {1: lambda: finish(1, 1, g[1, 1]),
                                                7: lambda: proj_q(7)})
                    g[1, 3] = attn_group(1, 3, {1: lambda: finish(1, 2, g[1, 2])})
                    finish(1, 3, g[1, 3])

                    if stop_after == "attn":
                        return
                    if with_collective:
                        nc.gpsimd.collective_compute(
                            "AllToAll", mybir.AluOpType.bypass,
                            replica_groups=[list(range(N_CORES))],
                            ins=[a2a_in.opt()], outs=[a2a_out.opt()])
                    else:
                        a2a_out = a2a_in  # timing-sim variant: skip collective

                    # ---- output projection (local tokens only) ----
                    ctxF = outp.tile([128, N_CORES, TSH], BF16)
                    for i in range(N_CORES):
                        nc.sync.dma_start(out=ctxF[:, i, :], in_=a2a_out[i])
                    for m in range(TSH // 128):
                        ob = outp.tile([128, C], F32, tag="ob", name="ob")
                        for half in range(2):
                            pso = ps.tile([128, 512], F32, tag="pss", name="pso")
                            for i in range(N_CORES):
                                nc.tensor.matmul(
                                    pso, lhsT=ctxF[:, i, m * 128:(m + 1) * 128],
                                    rhs=wo_sb[:, i, half * 512:(half + 1) * 512],
                                    start=(i == 0), stop=(i == N_CORES - 1))
                            nc.vector.tensor_add(ob[:, half * 512:(half + 1) * 512],
                                                 pso,
                                                 bias_sb[:, half * 512:(half + 1) * 512])
                        nc.sync.dma_start(out=out.ap()[m * 128:(m + 1) * 128, :], in_=ob)

            if reps is None:
                _body()
            else:
                with tc.For_i(0, reps, 1) as _it:
                    _body(_it)
    nc.compile()
    return nc


def _get_nc():
    global _NC_CACHE
    if _NC_CACHE is None:
        _NC_CACHE = _build()
    return _NC_CACHE


def prep_in_maps(query, key_value, w_q, w_kv, w_out, b_out):
    bf = ml_dtypes.bfloat16
    q2 = np.asarray(query, np.float32).reshape(T, C)
    kv2 = np.asarray(key_value, np.float32).reshape(T, C)
    qT_ = np.ascontiguousarray(q2.T).astype(bf)
    kvT_ = np.ascontiguousarray(kv2.T).astype(bf)
    wo_ = np.asarray(w_out, np.float32).astype(bf)
    bias = np.asarray(b_out, np.float32)

    in_maps = []
    for j in range(N_CORES):
        cs = slice(j * DL, (j + 1) * DL)
        in_maps.append({
            "qT": qT_,
            "kvT": kvT_,
            "wq": np.ascontiguousarray(np.asarray(w_q, np.float32)[:, cs]).astype(bf),
            "wk": np.ascontiguousarray(np.asarray(w_kv, np.float32)[:, cs]).astype(bf),
            "wv": np.ascontiguousarray(
                np.asarray(w_kv, np.float32)[:, C + j * DL: C + (j + 1) * DL]).astype(bf),
            "wo": wo_,
            "bias": bias,
        })
    return in_maps


def kernel(query, key_value, w_q, w_kv, w_out, b_out):
    global _LAST_RESULTS
    in_maps = prep_in_maps(query, key_value, w_q, w_kv, w_out, b_out)
    nc = _get_nc()
    res = run_bass_kernel_spmd(nc, in_maps, core_ids=list(range(N_CORES)))
    _LAST_RESULTS = res
    full = np.concatenate([res.results[j]["out"] for j in range(N_CORES)], axis=0)
    return full.reshape(B, NQ, C)
